# revision 6
# baseline (speedup 1.0000x reference)
"""Row-wise cosine similarity on 8 TRN2 NeuronCores.

out[n] = sum_d(p[n,d]*h[n,d]) / (max(||p[n]||,eps) * max(||h[n]||,eps))
with N=65536, D=1024, eps=1e-12 (torch F.normalize semantics).

Sharding: rows split evenly across 8 cores (data parallel, no comms).
Per core (8192 rows): rows are laid out as r = partition*64 + tile, so a
[128, G, 1024] SBUF tile loads G*4KB contiguous bytes per partition and
the per-row results land in a [128, 64] SBUF tile that DMAs out in one
contiguous-per-partition transfer (no on-chip transpose needed).

SDMA engine-79 derate: HW traces show SDMA engine 15 (trace id 79)
sustains only ~21.8 B/ns vs ~26.5 for engines 0-14 (periodic
half-rate bursts), and with single 128-partition transfers (8
descriptors per engine) it gates the whole input stream at ~349 GB/s.
The HWDGE deals a transfer's per-partition descriptors to engines by
filling each engine up to ceil(n_desc/16) in engine order, shorting
the trailing engines. Splitting each input load into partition ranges
[0:32]+[32:92]+[92:128] (32/60/28 descriptors) therefore deals 8
descriptors each to engines 0-13, 6 to engine 14, and 2 to engine 15:
the stream is paced by full-rate engines (~397 GB/s ceiling) and the
slow engine runs at 25% duty. Every dma_start delivers exactly +16 on
its semaphore regardless of partition count (HW-verified), and the
HWDGE queue is FIFO per engine, so a single semaphore on the last
split of each side covers the whole group.

Raw bass (no Tile scheduler): the walrus codegen in this toolchain
accepts at most ONE sync wait per instruction, which Tile's automatic
semaphore assignment violates for this dataflow. Hand-placed counting
semaphores keep every instruction at <=1 wait by exploiting
transitivity: the DVE group-completion increment happens after a wait
on the ACT group-completion sem, so DMAs recycling a buffer slot only
wait on the DVE sem.

Engine balance (HW-measured: ACT square+accum 1.41us, DVE
scalar_tensor_tensor+accum 1.28us per [128,1024] f32 tile): ACT
computes ||p||^2 everywhere plus one ||h||^2 per body group (keeping
both engines under the ~9.7us/group DMA period); in the taper the hh
columns alternate so both engines drain together right after the last
byte lands. Epilogue: ph * rsqrt(pp*hh) with ACT sqrt + DVE
reciprocal + one Newton-Raphson step.
"""

import numpy as np

try:
    import concourse.bass as bass
except ImportError:  # fresh grading dir: toolchain lives in /opt
    import sys

    sys.path.insert(0, "/opt/trn_rl_repo")
    import concourse.bass as bass

from contextlib import ExitStack

from concourse import mybir
from concourse.bass_utils import run_bass_kernel_spmd

N, D = 65536, 1024
NCORES = 8
ROWS = N // NCORES  # 8192 rows per core
P = 128  # SBUF partitions
T = ROWS // P  # 64 tile columns
GMAX = 4  # row-tiles per full group: one [128, G*1024] f32 load = 2 MB
B = 5  # in-flight group buffers
EPS2 = 1e-24  # eps^2; max(||x||,eps) == sqrt(max(||x||^2, eps^2)) here
# partition split points: 32/60/28 descriptors per dma_start deal as
# 8/8.../8, 6, 2 across the 16 SDMA engines (engine 15 is slow)
SPLITS = [(0, 32), (32, 92), (92, 128)]

_NC_CACHE = {}


def _group_sizes():
    """Slightly smaller first group (earlier compute start), full-size
    body, then a 2/2/1/1 taper so the post-stream compute drain is
    small."""
    sizes = [2] + [4] * 14 + [2, 2, 1, 1]
    assert sum(sizes) == T
    return sizes


def _hh_on_act(t):
    """Which columns' ||h||^2 runs on ACT instead of DVE: one per full
    body group keeps DVE under the DMA period; in the taper (58..61)
    alternate so both engines finish together; the last two columns
    stay on DVE (same-engine pipelining into the epilogue)."""
    return (t % 4 == 3 and t < 58) or t in (59, 61)


def _build_bass(detect_races=False):
    fp32 = mybir.dt.float32
    Sq = mybir.ActivationFunctionType.Square
    Sqrt = mybir.ActivationFunctionType.Sqrt
    mult = mybir.AluOpType.mult
    sizes = _group_sizes()
    starts = [sum(sizes[:i]) for i in range(len(sizes))]
    NG = len(sizes)

    nc = bass.Bass(detect_race_conditions=detect_races)
    prem = nc.declare_dram_parameter("premise", [ROWS, D], fp32, isOutput=False)
    hyp = nc.declare_dram_parameter("hypothesis", [ROWS, D], fp32, isOutput=False)
    outp = nc.declare_dram_parameter("out", [ROWS], fp32, isOutput=True)

    # row r = p*T + t: partition-strided input DMA, contiguous output DMA
    prem3 = prem[:].rearrange("(p t) d -> p t d", p=P)
    hyp3 = hyp[:].rearrange("(p t) d -> p t d", p=P)
    out2 = outp[:].rearrange("(p t) -> p t", p=P)

    with ExitStack() as mem:
        xs = [
            mem.enter_context(nc.sbuf_tensor(f"xs{i}", [P, 2, GMAX, D], fp32))
            for i in range(B)
        ]
        junk_a = mem.enter_context(nc.sbuf_tensor("junk_a", [P, D], fp32))
        junk_v = mem.enter_context(nc.sbuf_tensor("junk_v", [P, D], fp32))
        r_pp = mem.enter_context(nc.sbuf_tensor("r_pp", [P, T], fp32))
        r_hh = mem.enter_context(nc.sbuf_tensor("r_hh", [P, T], fp32))
        r_ph = mem.enter_context(nc.sbuf_tensor("r_ph", [P, T], fp32))
        d2 = mem.enter_context(nc.sbuf_tensor("d2", [P, T], fp32))
        sd = mem.enter_context(nc.sbuf_tensor("sd", [P, T], fp32))
        yv = mem.enter_context(nc.sbuf_tensor("yv", [P, T], fp32))
        t1 = mem.enter_context(nc.sbuf_tensor("t1", [P, T], fp32))
        res = mem.enter_context(nc.sbuf_tensor("res", [P, T], fp32))

        with ExitStack() as semctx:
            s_dma_p = [
                semctx.enter_context(nc.semaphore(f"s_dma_p{i}")) for i in range(8)
            ]
            s_dma_h = [
                semctx.enter_context(nc.semaphore(f"s_dma_h{i}")) for i in range(8)
            ]
            s_act = semctx.enter_context(nc.semaphore("s_act"))
            s_dve = semctx.enter_context(nc.semaphore("s_dve"))
            s_ch = semctx.enter_context(nc.semaphore("s_ch"))
            s_ep2 = semctx.enter_context(nc.semaphore("s_ep2"))
            s_res = semctx.enter_context(nc.semaphore("s_res"))
            s_out = semctx.enter_context(nc.semaphore("s_out"))
            # codegen requires sync info on every dynamic DMA; the
            # non-final splits dump their +16 here, nobody waits on it
            s_trash = semctx.enter_context(nc.semaphore("s_trash"))

            def issue_side(eng, g, side, src3, sem):
                s0, g0 = starts[g], sizes[g]
                for k, (pa, pb) in enumerate(SPLITS):
                    tgt = sem if k == len(SPLITS) - 1 else s_trash
                    eng.dma_start(
                        out=xs[g % B][pa:pb, side, :g0, :],
                        in_=src3[pa:pb, s0 : s0 + g0, :],
                    ).then_inc(tgt, 16)

            with nc.Block() as block:

                @block.sync
                def _(eng: bass.BassEngine):
                    for g in range(NG):
                        if g >= B:
                            # DVE inc implies ACT done too (transitive)
                            eng.wait_ge(s_dve, g - B + 1)
                        issue_side(eng, g, 0, prem3, s_dma_p[g % 8])
                        issue_side(eng, g, 1, hyp3, s_dma_h[g % 8])
                    eng.wait_ge(s_res, 1)
                    eng.dma_start(out=out2, in_=res[:]).then_inc(s_out, 16)
                    eng.wait_ge(s_out, 16)

                @block.scalar
                def _(eng: bass.BassEngine):
                    for g in range(NG):
                        eng.wait_ge(s_dma_p[g % 8], 16 * (g // 8 + 1))
                        sl = xs[g % B]
                        hh_mine = [
                            j for j in range(sizes[g]) if _hh_on_act(starts[g] + j)
                        ]
                        last = ("hh", hh_mine[-1]) if hh_mine else ("pp", sizes[g] - 1)
                        for j in range(sizes[g]):
                            t = starts[g] + j
                            ins = eng.activation(
                                out=junk_a[:, :],
                                in_=sl[:, 0, j, :],
                                func=Sq,
                                accum_out=r_pp[:, t : t + 1],
                            )
                            if last == ("pp", j):
                                ins.then_inc(s_act, 1)
                        if hh_mine:
                            eng.wait_ge(s_dma_h[g % 8], 16 * (g // 8 + 1))
                            for j in hh_mine:
                                t = starts[g] + j
                                ins = eng.activation(
                                    out=junk_a[:, :],
                                    in_=sl[:, 1, j, :],
                                    func=Sq,
                                    accum_out=r_hh[:, t : t + 1],
                                )
                                if last == ("hh", j):
                                    ins.then_inc(s_act, 1)
                    # epilogue: sqrt of pp*hh (after DVE built d2 = 2nd chain inc)
                    eng.wait_ge(s_ch, 2)
                    eng.activation(out=sd[:], in_=d2[:], func=Sqrt).then_inc(s_ep2, 1)

                @block.vector
                def _(eng: bass.BassEngine):
                    mx = mybir.AluOpType.max
                    add = mybir.AluOpType.add
                    for g in range(NG):
                        # the h side's final split is issued after every
                        # other split of both sides on the same FIFO
                        # queue: one wait covers the whole group.
                        eng.wait_ge(s_dma_h[g % 8], 16 * (g // 8 + 1))
                        sl = xs[g % B]
                        ops = []  # (kind, j, t)
                        for j in range(sizes[g]):
                            t = starts[g] + j
                            ops.append(("ph", j, t))
                            if not _hh_on_act(t):
                                ops.append(("hh", j, t))
                        for k, (kind, j, t) in enumerate(ops):
                            if k == len(ops) - 1:
                                # DVE completion of group g implies ACT done
                                eng.wait_ge(s_act, g + 1)
                            side0 = 0 if kind == "ph" else 1
                            accum = r_ph if kind == "ph" else r_hh
                            ins = eng.scalar_tensor_tensor(
                                out=junk_v[:, :],
                                in0=sl[:, side0, j, :],
                                scalar=1.0,
                                in1=sl[:, 1, j, :],
                                op0=mult,
                                op1=mult,
                                accum_out=accum[:, t : t + 1],
                            )
                            if k == len(ops) - 1:
                                ins.then_inc(s_dve, 1)
                    # epilogue: res = ph * rsqrt(max(pp,e)*max(hh,e)).
                    # DVE pipelines same-engine dependent ops, so every
                    # same-engine RAW needs a sem; s_ch counts epilogue
                    # DVE completions. s_act >= NG was observed in the
                    # g-loop; s_dve >= NG orders the r_ph/r_hh reads
                    # behind the last stt writes.
                    eng.wait_ge(s_dve, NG)
                    eng.tensor_scalar_max(
                        out=r_hh[:], in0=r_hh[:], scalar1=EPS2
                    ).then_inc(s_ch, 1)
                    eng.wait_ge(s_ch, 1)
                    eng.scalar_tensor_tensor(
                        out=d2[:], in0=r_pp[:], scalar=EPS2, in1=r_hh[:],
                        op0=mx, op1=mult,
                    ).then_inc(s_ch, 1)
                    eng.wait_ge(s_ep2, 1)
                    eng.reciprocal(out=yv[:], in_=sd[:]).then_inc(s_ch, 1)
                    # Newton step for rsqrt: y *= 1.5 - 0.5*d2*y*y
                    eng.wait_ge(s_ch, 3)
                    eng.tensor_mul(t1[:], yv[:], yv[:]).then_inc(s_ch, 1)
                    eng.wait_ge(s_ch, 4)
                    eng.scalar_tensor_tensor(
                        out=t1[:], in0=d2[:], scalar=-0.5, in1=t1[:],
                        op0=mult, op1=mult,
                    ).then_inc(s_ch, 1)
                    eng.wait_ge(s_ch, 5)
                    eng.scalar_tensor_tensor(
                        out=yv[:], in0=t1[:], scalar=1.5, in1=yv[:],
                        op0=add, op1=mult,
                    ).then_inc(s_ch, 1)
                    eng.wait_ge(s_ch, 6)
                    eng.tensor_mul(res[:], r_ph[:], yv[:]).then_inc(s_res, 1)

    return nc


def _get_nc():
    if "nc" not in _NC_CACHE:
        _NC_CACHE["nc"] = _build_bass()
    return _NC_CACHE["nc"]


def _run(premise, hypothesis, trace=False, **kwargs):
    premise = np.ascontiguousarray(np.asarray(premise, dtype=np.float32))
    hypothesis = np.ascontiguousarray(np.asarray(hypothesis, dtype=np.float32))
    assert premise.shape == (N, D) and hypothesis.shape == (N, D)
    nc = _get_nc()
    in_maps = [
        {
            "premise": premise[c * ROWS : (c + 1) * ROWS],
            "hypothesis": hypothesis[c * ROWS : (c + 1) * ROWS],
        }
        for c in range(NCORES)
    ]
    r = run_bass_kernel_spmd(nc, in_maps, list(range(NCORES)), trace=trace, **kwargs)
    out = np.concatenate([r.results[c]["out"] for c in range(NCORES)])
    return out, r


def kernel(premise, hypothesis):
    out, _ = _run(premise, hypothesis)
    return out


# revision 9
# speedup vs baseline: 1.5839x; 1.5839x over previous
"""Row-wise cosine similarity on 8 TRN2 NeuronCores.

out[n] = sum_d(p[n,d]*h[n,d]) / (max(||p[n]||,eps) * max(||h[n]||,eps))
with N=65536, D=1024, eps=1e-12 (torch F.normalize semantics).

Sharding: rows split evenly across 8 cores (data parallel, no comms).
Per core (8192 rows): rows are laid out as r = partition*64 + tile, so a
[128, G, 1024] SBUF tile loads G*4KB contiguous bytes per partition and
the per-row results land in a [128, 64] SBUF tile that DMAs out in one
contiguous-per-partition transfer (no on-chip transpose needed).

Raw bass (no Tile scheduler): the walrus codegen in this toolchain
accepts at most ONE sync wait per instruction, which Tile's automatic
semaphore assignment violates for this dataflow. Hand-placed counting
semaphores keep every instruction at <=1 wait by exploiting transitivity:
the DVE group-completion increment happens after a wait on the ACT
group-completion sem, so DMAs recycling a buffer slot only wait on the
DVE sem.

Engine balance (HW-measured: ACTIVATE+accum-read 1.41us, DVE
scalar_tensor_tensor+accum 1.29us per [128,1024] tile): ACT computes
||p||^2 (Square + row-accumulate), DVE computes p.h and ||h||^2 via
scalar_tensor_tensor accum_out, with the last few tiles' ||h||^2 on
ACT so DVE drains quickly after the final DMA. All input loads issue
from the SP HWDGE ring; group sizes taper at both ends. Epilogue: ph * rsqrt(pp*hh) with ACT sqrt + DVE reciprocal + one
Newton-Raphson step.
"""

import numpy as np

try:
    import concourse.bass as bass
except ImportError:  # fresh grading dir: toolchain lives in /opt
    import sys

    sys.path.insert(0, "/opt/trn_rl_repo")
    import concourse.bass as bass

from contextlib import ExitStack

from concourse import mybir
from concourse.bass_utils import run_bass_kernel_spmd

N, D = 65536, 1024
NCORES = 8
ROWS = N // NCORES  # 8192 rows per core
P = 128  # SBUF partitions
GMAX = 8  # row-tiles per full group: one [128, G*1024] f32 load = 4 MB
B = 3  # in-flight group buffers (3 x 64KB/partition = 192KB of 208KB)
EPS2 = 1e-24  # eps^2; max(||x||,eps) == sqrt(max(||x||^2, eps^2)) here

_NC_CACHE = {}


def _group_sizes(T, gmax):
    """Slightly smaller first group (earlier compute start without
    starving the DMA issue pipeline), full-size body, then a short taper
    so the post-stream compute drain is small."""
    front = [t for t in (2,) if t < gmax]
    back = [t for t in (2, 1, 1) if t < gmax]
    body = T - sum(front) - sum(back)
    sizes = [gmax] * (body // gmax)
    rem = body % gmax
    if rem:
        sizes.append(rem)
    sizes = front + sizes + back
    assert sum(sizes) == T
    return sizes


def _hh_on_act(t, T):
    """Which tiles' ||h||^2 runs on ACT instead of DVE: two per body
    group of 8 keeps DVE under the DMA period; in the taper (T-6..T-3)
    alternate so both engines drain together; the last two tiles stay
    on DVE (same-engine pipelining into the epilogue)."""
    return (t % 4 == 3 and t < T - 6) or t in (T - 5, T - 3)


def _build_bass(rows=ROWS, gmax=GMAX, b=B, unique_junk=False, detect_races=False,
                taper=True):
    fp32 = mybir.dt.float32
    Sq = mybir.ActivationFunctionType.Square
    Sqrt = mybir.ActivationFunctionType.Sqrt
    mult = mybir.AluOpType.mult
    T = rows // P
    B = b
    sizes = _group_sizes(T, gmax) if taper else [gmax] * (T // gmax)
    starts = [sum(sizes[:i]) for i in range(len(sizes))]
    NG = len(sizes)

    nc = bass.Bass(detect_race_conditions=detect_races)
    prem = nc.declare_dram_parameter("premise", [rows, D], fp32, isOutput=False)
    hyp = nc.declare_dram_parameter("hypothesis", [rows, D], fp32, isOutput=False)
    outp = nc.declare_dram_parameter("out", [rows], fp32, isOutput=True)

    # row r = p*T + t: partition-strided input DMA, contiguous output DMA
    prem3 = prem[:].rearrange("(p t) d -> p t d", p=P)
    hyp3 = hyp[:].rearrange("(p t) d -> p t d", p=P)
    out2 = outp[:].rearrange("(p t) -> p t", p=P)

    # junk: mandatory full-size outputs of accumulate ops; values unused.
    # unique_junk gives every instruction its own slice (race-detector-clean
    # validation builds only — too big for the full problem size).
    na = 2 * T if unique_junk else 1
    nv = 2 * T if unique_junk else 1

    with ExitStack() as mem:
        xs = [
            mem.enter_context(nc.sbuf_tensor(f"xs{i}", [P, 2, gmax, D], fp32))
            for i in range(B)
        ]
        junk_a = mem.enter_context(nc.sbuf_tensor("junk_a", [P, na, D], fp32))
        junk_v = mem.enter_context(nc.sbuf_tensor("junk_v", [P, nv, D], fp32))
        r_pp = mem.enter_context(nc.sbuf_tensor("r_pp", [P, T], fp32))
        r_hh = mem.enter_context(nc.sbuf_tensor("r_hh", [P, T], fp32))
        r_ph = mem.enter_context(nc.sbuf_tensor("r_ph", [P, T], fp32))
        d2 = mem.enter_context(nc.sbuf_tensor("d2", [P, T], fp32))
        sd = mem.enter_context(nc.sbuf_tensor("sd", [P, T], fp32))
        yv = mem.enter_context(nc.sbuf_tensor("yv", [P, T], fp32))
        t1 = mem.enter_context(nc.sbuf_tensor("t1", [P, T], fp32))
        res = mem.enter_context(nc.sbuf_tensor("res", [P, T], fp32))

        with ExitStack() as semctx:
            s_dma_p = [
                semctx.enter_context(nc.semaphore(f"s_dma_p{i}")) for i in range(8)
            ]
            s_dma_h = [
                semctx.enter_context(nc.semaphore(f"s_dma_h{i}")) for i in range(8)
            ]
            s_act = semctx.enter_context(nc.semaphore("s_act"))
            s_dve = semctx.enter_context(nc.semaphore("s_dve"))
            s_ch = semctx.enter_context(nc.semaphore("s_ch"))
            s_ep2 = semctx.enter_context(nc.semaphore("s_ep2"))
            s_res = semctx.enter_context(nc.semaphore("s_res"))
            s_out = semctx.enter_context(nc.semaphore("s_out"))

            def pslice(g):
                s0, g0 = starts[g], sizes[g]
                return prem3[:, s0 : s0 + g0, :]

            def hslice(g):
                s0, g0 = starts[g], sizes[g]
                return hyp3[:, s0 : s0 + g0, :]

            with nc.Block() as block:

                @block.sync
                def _(eng: bass.BassEngine):
                    for g in range(NG):
                        if g >= B:
                            # DVE inc implies ACT done too (transitive)
                            eng.wait_ge(s_dve, g - B + 1)
                        eng.dma_start(
                            out=xs[g % B][:, 0, : sizes[g], :], in_=pslice(g)
                        ).then_inc(s_dma_p[g % 8], 16)
                        eng.dma_start(
                            out=xs[g % B][:, 1, : sizes[g], :], in_=hslice(g)
                        ).then_inc(s_dma_h[g % 8], 16)
                    eng.wait_ge(s_res, 1)
                    eng.dma_start(out=out2, in_=res[:]).then_inc(s_out, 16)
                    eng.wait_ge(s_out, 16)

                @block.scalar
                def _(eng: bass.BassEngine):
                    for g in range(NG):
                        eng.wait_ge(s_dma_p[g % 8], 16 * (g // 8 + 1))
                        sl = xs[g % B]
                        hh_mine = [
                            j for j in range(sizes[g]) if _hh_on_act(starts[g] + j, T)
                        ]
                        last = ("hh", hh_mine[-1]) if hh_mine else ("pp", sizes[g] - 1)
                        for j in range(sizes[g]):
                            t = starts[g] + j
                            ins = eng.activation(
                                out=junk_a[:, t % na, :],
                                in_=sl[:, 0, j, :],
                                func=Sq,
                                accum_out=r_pp[:, t : t + 1],
                            )
                            if last == ("pp", j):
                                ins.then_inc(s_act, 1)
                        if hh_mine:
                            eng.wait_ge(s_dma_h[g % 8], 16 * (g // 8 + 1))
                            for j in hh_mine:
                                t = starts[g] + j
                                ins = eng.activation(
                                    out=junk_a[:, (t + T) % na, :],
                                    in_=sl[:, 1, j, :],
                                    func=Sq,
                                    accum_out=r_hh[:, t : t + 1],
                                )
                                if last == ("hh", j):
                                    ins.then_inc(s_act, 1)
                    # epilogue: sqrt of pp*hh (after DVE built d2 = 2nd chain inc)
                    eng.wait_ge(s_ch, 2)
                    eng.activation(out=sd[:], in_=d2[:], func=Sqrt).then_inc(s_ep2, 1)

                @block.vector
                def _(eng: bass.BassEngine):
                    mx = mybir.AluOpType.max
                    add = mybir.AluOpType.add
                    for g in range(NG):
                        # p(g) and h(g) are issued back-to-back on the same
                        # SP HWDGE queue; each SDMA engine drains its ring
                        # FIFO, so 16 incs on the h sem imply p landed too.
                        eng.wait_ge(s_dma_h[g % 8], 16 * (g // 8 + 1))
                        sl = xs[g % B]
                        ops = []  # (kind, j, t)
                        for j in range(sizes[g]):
                            t = starts[g] + j
                            ops.append(("ph", j, t))
                            if not _hh_on_act(t, T):
                                ops.append(("hh", j, t))
                        for k, (kind, j, t) in enumerate(ops):
                            if k == len(ops) - 1:
                                # DVE completion of group g implies ACT done
                                eng.wait_ge(s_act, g + 1)
                            if kind == "ph":
                                ins = eng.scalar_tensor_tensor(
                                    out=junk_v[:, (2 * t) % nv, :],
                                    in0=sl[:, 0, j, :],
                                    scalar=1.0,
                                    in1=sl[:, 1, j, :],
                                    op0=mult,
                                    op1=mult,
                                    accum_out=r_ph[:, t : t + 1],
                                )
                            else:
                                ins = eng.scalar_tensor_tensor(
                                    out=junk_v[:, (2 * t + 1) % nv, :],
                                    in0=sl[:, 1, j, :],
                                    scalar=1.0,
                                    in1=sl[:, 1, j, :],
                                    op0=mult,
                                    op1=mult,
                                    accum_out=r_hh[:, t : t + 1],
                                )
                            if k == len(ops) - 1:
                                ins.then_inc(s_dve, 1)
                    # epilogue: res = ph * rsqrt(max(pp,e)*max(hh,e)).
                    # DVE pipelines same-engine dependent ops, so every
                    # same-engine RAW needs a sem; s_ch counts epilogue
                    # DVE completions. s_act >= NG was observed in the
                    # g-loop; s_dve >= NG orders the r_ph/r_hh reads
                    # behind the last stt writes.
                    eng.wait_ge(s_dve, NG)
                    eng.tensor_scalar_max(
                        out=r_hh[:], in0=r_hh[:], scalar1=EPS2
                    ).then_inc(s_ch, 1)
                    eng.wait_ge(s_ch, 1)
                    eng.scalar_tensor_tensor(
                        out=d2[:], in0=r_pp[:], scalar=EPS2, in1=r_hh[:],
                        op0=mx, op1=mult,
                    ).then_inc(s_ch, 1)
                    eng.wait_ge(s_ep2, 1)
                    eng.reciprocal(out=yv[:], in_=sd[:]).then_inc(s_ch, 1)
                    # Newton step for rsqrt: y *= 1.5 - 0.5*d2*y*y
                    eng.wait_ge(s_ch, 3)
                    eng.tensor_mul(t1[:], yv[:], yv[:]).then_inc(s_ch, 1)
                    eng.wait_ge(s_ch, 4)
                    eng.scalar_tensor_tensor(
                        out=t1[:], in0=d2[:], scalar=-0.5, in1=t1[:],
                        op0=mult, op1=mult,
                    ).then_inc(s_ch, 1)
                    eng.wait_ge(s_ch, 5)
                    eng.scalar_tensor_tensor(
                        out=yv[:], in0=t1[:], scalar=1.5, in1=yv[:],
                        op0=add, op1=mult,
                    ).then_inc(s_ch, 1)
                    eng.wait_ge(s_ch, 6)
                    eng.tensor_mul(res[:], r_ph[:], yv[:]).then_inc(s_res, 1)

    return nc


def _get_nc():
    if "nc" not in _NC_CACHE:
        _NC_CACHE["nc"] = _build_bass()
    return _NC_CACHE["nc"]


def _run(premise, hypothesis, trace=False, **kwargs):
    premise = np.ascontiguousarray(np.asarray(premise, dtype=np.float32))
    hypothesis = np.ascontiguousarray(np.asarray(hypothesis, dtype=np.float32))
    assert premise.shape == (N, D) and hypothesis.shape == (N, D)
    nc = _get_nc()
    in_maps = [
        {
            "premise": premise[c * ROWS : (c + 1) * ROWS],
            "hypothesis": hypothesis[c * ROWS : (c + 1) * ROWS],
        }
        for c in range(NCORES)
    ]
    r = run_bass_kernel_spmd(nc, in_maps, list(range(NCORES)), trace=trace, **kwargs)
    out = np.concatenate([r.results[c]["out"] for c in range(NCORES)])
    return out, r


def kernel(premise, hypothesis):
    out, _ = _run(premise, hypothesis)
    return out



# revision 16
# speedup vs baseline: 1.6248x; 1.0259x over previous
"""Row-wise cosine similarity on 8 TRN2 NeuronCores.

out[n] = sum_d(p[n,d]*h[n,d]) / (max(||p[n]||,eps) * max(||h[n]||,eps))
with N=65536, D=1024, eps=1e-12 (torch F.normalize semantics).

Sharding: rows split evenly across 8 cores (data parallel, no comms).
Per core (8192 rows): rows are laid out as r = partition*64 + tile, so a
[128, G, 1024] SBUF tile loads G*4KB contiguous bytes per partition and
the per-row results land in a [128, 64] SBUF tile that DMAs out in one
contiguous-per-partition transfer (no on-chip transpose needed).

Raw bass (no Tile scheduler): the walrus codegen in this toolchain
accepts at most ONE sync wait per instruction, which Tile's automatic
semaphore assignment violates for this dataflow. Hand-placed counting
semaphores keep every instruction at <=1 wait by exploiting transitivity:
the DVE group-completion increment happens after a wait on the ACT
group-completion sem, so DMAs recycling a buffer slot only wait on the
DVE sem.

Engine balance (HW-measured: ACTIVATE+accum-read 1.41us, DVE
scalar_tensor_tensor+accum 1.29us per [128,1024] tile): ACT computes
||p||^2 (Square + row-accumulate), DVE computes p.h and ||h||^2 via
scalar_tensor_tensor accum_out, with the last few tiles' ||h||^2 on
ACT so DVE drains quickly after the final DMA. All input loads issue
from the SP HWDGE ring; group sizes taper at both ends. Epilogue: ph * rsqrt(pp*hh) with ACT sqrt + DVE reciprocal + one
Newton-Raphson step.
"""

import numpy as np

try:
    import concourse.bass as bass
except ImportError:  # fresh grading dir: toolchain lives in /opt
    import sys

    sys.path.insert(0, "/opt/trn_rl_repo")
    import concourse.bass as bass

from contextlib import ExitStack

from concourse import mybir
from concourse.bass_utils import run_bass_kernel_spmd

N, D = 65536, 1024
NCORES = 8
ROWS = N // NCORES  # 8192 rows per core
P = 128  # SBUF partitions
GMAX = 4  # row-tiles per full group: one [128, G*1024] f32 load = 2 MB
B = 5  # in-flight group buffers
EPS2 = 1e-24  # eps^2; max(||x||,eps) == sqrt(max(||x||^2, eps^2)) here

_NC_CACHE = {}


def _group_sizes(T, gmax):
    """Slightly smaller first group (earlier compute start without
    starving the DMA issue pipeline), full-size body, then a short taper
    so the post-stream compute drain is small."""
    front = [t for t in (2,) if t < gmax]
    back = [t for t in (2, 1, 1) if t < gmax]
    body = T - sum(front) - sum(back)
    sizes = [gmax] * (body // gmax)
    rem = body % gmax
    if rem:
        sizes.append(rem)
    sizes = front + sizes + back
    assert sum(sizes) == T
    return sizes


def _hh_on_act(t, T):
    """Which tiles' ||h||^2 runs on ACT instead of DVE: one per body
    group of 4 keeps DVE comfortably under the DMA period throughout
    (instead of piling 8 tail tiles onto ACT, which made ACT the
    post-stream drain bottleneck); in the taper the hh columns
    alternate so both engines finish together; the last two stay on
    DVE (same-engine pipelining into the epilogue)."""
    return (t % 4 == 3 and t < T - 6) or t in (T - 5, T - 3)


def _build_bass(rows=ROWS, gmax=GMAX, b=B, unique_junk=False, detect_races=False,
                taper=True):
    fp32 = mybir.dt.float32
    Sq = mybir.ActivationFunctionType.Square
    Sqrt = mybir.ActivationFunctionType.Sqrt
    mult = mybir.AluOpType.mult
    T = rows // P
    B = b
    sizes = _group_sizes(T, gmax) if taper else [gmax] * (T // gmax)
    starts = [sum(sizes[:i]) for i in range(len(sizes))]
    NG = len(sizes)

    nc = bass.Bass(detect_race_conditions=detect_races)
    prem = nc.declare_dram_parameter("premise", [rows, D], fp32, isOutput=False)
    hyp = nc.declare_dram_parameter("hypothesis", [rows, D], fp32, isOutput=False)
    outp = nc.declare_dram_parameter("out", [rows], fp32, isOutput=True)

    # row r = p*T + t: partition-strided input DMA, contiguous output DMA
    prem3 = prem[:].rearrange("(p t) d -> p t d", p=P)
    hyp3 = hyp[:].rearrange("(p t) d -> p t d", p=P)
    out2 = outp[:].rearrange("(p t) -> p t", p=P)

    # junk: mandatory full-size outputs of accumulate ops; values unused.
    # unique_junk gives every instruction its own slice (race-detector-clean
    # validation builds only — too big for the full problem size).
    na = 2 * T if unique_junk else 1
    nv = 2 * T if unique_junk else 1

    with ExitStack() as mem:
        xs = [
            mem.enter_context(nc.sbuf_tensor(f"xs{i}", [P, 2, gmax, D], fp32))
            for i in range(B)
        ]
        junk_a = mem.enter_context(nc.sbuf_tensor("junk_a", [P, na, D], fp32))
        junk_v = mem.enter_context(nc.sbuf_tensor("junk_v", [P, nv, D], fp32))
        r_pp = mem.enter_context(nc.sbuf_tensor("r_pp", [P, T], fp32))
        r_hh = mem.enter_context(nc.sbuf_tensor("r_hh", [P, T], fp32))
        r_ph = mem.enter_context(nc.sbuf_tensor("r_ph", [P, T], fp32))
        d2 = mem.enter_context(nc.sbuf_tensor("d2", [P, T], fp32))
        sd = mem.enter_context(nc.sbuf_tensor("sd", [P, T], fp32))
        yv = mem.enter_context(nc.sbuf_tensor("yv", [P, T], fp32))
        t1 = mem.enter_context(nc.sbuf_tensor("t1", [P, T], fp32))
        res = mem.enter_context(nc.sbuf_tensor("res", [P, T], fp32))

        with ExitStack() as semctx:
            s_dma_p = [
                semctx.enter_context(nc.semaphore(f"s_dma_p{i}")) for i in range(8)
            ]
            s_dma_h = [
                semctx.enter_context(nc.semaphore(f"s_dma_h{i}")) for i in range(8)
            ]
            s_act = semctx.enter_context(nc.semaphore("s_act"))
            s_dve = semctx.enter_context(nc.semaphore("s_dve"))
            s_ch = semctx.enter_context(nc.semaphore("s_ch"))
            s_ep2 = semctx.enter_context(nc.semaphore("s_ep2"))
            s_res = semctx.enter_context(nc.semaphore("s_res"))
            s_out = semctx.enter_context(nc.semaphore("s_out"))

            def pslice(g):
                s0, g0 = starts[g], sizes[g]
                return prem3[:, s0 : s0 + g0, :]

            def hslice(g):
                s0, g0 = starts[g], sizes[g]
                return hyp3[:, s0 : s0 + g0, :]

            # Epilogue is split in two column halves: the first HALF
            # columns' rsqrt chain runs mid-stream (right after group
            # GA-1, whose results it needs, using DVE/ACT slack under
            # the DMA period), so only the last T-HALF columns' chain
            # remains in the post-stream drain.
            GA = min(14, NG)
            HALF = starts[GA] if GA < NG else T

            with nc.Block() as block:

                @block.sync
                def _(eng: bass.BassEngine):
                    for g in range(NG):
                        if g >= B:
                            # DVE inc implies ACT done too (transitive)
                            eng.wait_ge(s_dve, g - B + 1)
                        eng.dma_start(
                            out=xs[g % B][:, 0, : sizes[g], :], in_=pslice(g)
                        ).then_inc(s_dma_p[g % 8], 16)
                        eng.dma_start(
                            out=xs[g % B][:, 1, : sizes[g], :], in_=hslice(g)
                        ).then_inc(s_dma_h[g % 8], 16)
                    eng.wait_ge(s_res, 2)
                    eng.dma_start(out=out2, in_=res[:]).then_inc(s_out, 16)
                    eng.wait_ge(s_out, 16)

                @block.scalar
                def _(eng: bass.BassEngine):
                    for g in range(NG):
                        eng.wait_ge(s_dma_p[g % 8], 16 * (g // 8 + 1))
                        sl = xs[g % B]
                        hh_mine = [
                            j for j in range(sizes[g]) if _hh_on_act(starts[g] + j, T)
                        ]
                        last = ("hh", hh_mine[-1]) if hh_mine else ("pp", sizes[g] - 1)
                        for j in range(sizes[g]):
                            t = starts[g] + j
                            ins = eng.activation(
                                out=junk_a[:, t % na, :],
                                in_=sl[:, 0, j, :],
                                func=Sq,
                                accum_out=r_pp[:, t : t + 1],
                            )
                            if last == ("pp", j):
                                ins.then_inc(s_act, 1)
                        if hh_mine:
                            eng.wait_ge(s_dma_h[g % 8], 16 * (g // 8 + 1))
                            for j in hh_mine:
                                t = starts[g] + j
                                ins = eng.activation(
                                    out=junk_a[:, (t + T) % na, :],
                                    in_=sl[:, 1, j, :],
                                    func=Sq,
                                    accum_out=r_hh[:, t : t + 1],
                                )
                                if last == ("hh", j):
                                    ins.then_inc(s_act, 1)
                        if g == GA - 1:
                            # first-half sqrt (after DVE built d2[:HALF])
                            eng.wait_ge(s_ch, 2)
                            eng.activation(
                                out=sd[:, :HALF], in_=d2[:, :HALF], func=Sqrt
                            ).then_inc(s_ep2, 1)
                    # second-half sqrt (after DVE built d2[HALF:])
                    eng.wait_ge(s_ch, 8)
                    eng.activation(
                        out=sd[:, HALF:], in_=d2[:, HALF:], func=Sqrt
                    ).then_inc(s_ep2, 1)

                @block.vector
                def _(eng: bass.BassEngine):
                    mx = mybir.AluOpType.max
                    add = mybir.AluOpType.add

                    def epi(lo, hi, dve_thresh, ch0, ep):
                        # res[:, lo:hi] = ph * rsqrt(max(pp,e)*max(hh,e)).
                        # DVE pipelines same-engine dependent ops, so
                        # every same-engine RAW needs a sem; s_ch counts
                        # epilogue DVE completions. s_dve >= dve_thresh
                        # orders the r_ph/r_hh reads behind the group
                        # stt accum writes.
                        eng.wait_ge(s_dve, dve_thresh)
                        eng.tensor_scalar_max(
                            out=r_hh[:, lo:hi], in0=r_hh[:, lo:hi], scalar1=EPS2
                        ).then_inc(s_ch, 1)
                        eng.wait_ge(s_ch, ch0 + 1)
                        eng.scalar_tensor_tensor(
                            out=d2[:, lo:hi], in0=r_pp[:, lo:hi], scalar=EPS2,
                            in1=r_hh[:, lo:hi], op0=mx, op1=mult,
                        ).then_inc(s_ch, 1)
                        eng.wait_ge(s_ep2, ep)
                        eng.reciprocal(out=yv[:, lo:hi], in_=sd[:, lo:hi]).then_inc(
                            s_ch, 1
                        )
                        # Newton step for rsqrt: y *= 1.5 - 0.5*d2*y*y
                        eng.wait_ge(s_ch, ch0 + 3)
                        eng.tensor_mul(
                            t1[:, lo:hi], yv[:, lo:hi], yv[:, lo:hi]
                        ).then_inc(s_ch, 1)
                        eng.wait_ge(s_ch, ch0 + 4)
                        eng.scalar_tensor_tensor(
                            out=t1[:, lo:hi], in0=d2[:, lo:hi], scalar=-0.5,
                            in1=t1[:, lo:hi], op0=mult, op1=mult,
                        ).then_inc(s_ch, 1)
                        eng.wait_ge(s_ch, ch0 + 5)
                        eng.scalar_tensor_tensor(
                            out=yv[:, lo:hi], in0=t1[:, lo:hi], scalar=1.5,
                            in1=yv[:, lo:hi], op0=add, op1=mult,
                        ).then_inc(s_ch, 1)
                        eng.wait_ge(s_ch, ch0 + 6)
                        eng.tensor_mul(
                            res[:, lo:hi], r_ph[:, lo:hi], yv[:, lo:hi]
                        ).then_inc(s_res, 1)

                    for g in range(NG):
                        # p(g) and h(g) are issued back-to-back on the same
                        # SP HWDGE queue; each SDMA engine drains its ring
                        # FIFO, so 16 incs on the h sem imply p landed too.
                        eng.wait_ge(s_dma_h[g % 8], 16 * (g // 8 + 1))
                        sl = xs[g % B]
                        ops = []  # (kind, j, t)
                        for j in range(sizes[g]):
                            t = starts[g] + j
                            ops.append(("ph", j, t))
                            if not _hh_on_act(t, T):
                                ops.append(("hh", j, t))
                        for k, (kind, j, t) in enumerate(ops):
                            if k == len(ops) - 1:
                                # DVE completion of group g implies ACT done
                                eng.wait_ge(s_act, g + 1)
                            if kind == "ph":
                                ins = eng.scalar_tensor_tensor(
                                    out=junk_v[:, (2 * t) % nv, :],
                                    in0=sl[:, 0, j, :],
                                    scalar=1.0,
                                    in1=sl[:, 1, j, :],
                                    op0=mult,
                                    op1=mult,
                                    accum_out=r_ph[:, t : t + 1],
                                )
                            else:
                                ins = eng.scalar_tensor_tensor(
                                    out=junk_v[:, (2 * t + 1) % nv, :],
                                    in0=sl[:, 1, j, :],
                                    scalar=1.0,
                                    in1=sl[:, 1, j, :],
                                    op0=mult,
                                    op1=mult,
                                    accum_out=r_hh[:, t : t + 1],
                                )
                            if k == len(ops) - 1:
                                ins.then_inc(s_dve, 1)
                        if g == GA - 1:
                            epi(0, HALF, GA, 0, 1)
                    epi(HALF, T, NG, 6, 2)

    return nc


def _get_nc():
    if "nc" not in _NC_CACHE:
        _NC_CACHE["nc"] = _build_bass()
    return _NC_CACHE["nc"]


def _run(premise, hypothesis, trace=False, **kwargs):
    premise = np.ascontiguousarray(np.asarray(premise, dtype=np.float32))
    hypothesis = np.ascontiguousarray(np.asarray(hypothesis, dtype=np.float32))
    assert premise.shape == (N, D) and hypothesis.shape == (N, D)
    nc = _get_nc()
    in_maps = [
        {
            "premise": premise[c * ROWS : (c + 1) * ROWS],
            "hypothesis": hypothesis[c * ROWS : (c + 1) * ROWS],
        }
        for c in range(NCORES)
    ]
    r = run_bass_kernel_spmd(nc, in_maps, list(range(NCORES)), trace=trace, **kwargs)
    out = np.concatenate([r.results[c]["out"] for c in range(NCORES)])
    return out, r


def kernel(premise, hypothesis):
    out, _ = _run(premise, hypothesis)
    return out

